# revision 8
# baseline (speedup 1.0000x reference)
"""RBF kernel layer (retrieval_knn): out = exp(-||x - p||^2) for x [131072, 64]
against 512 prototypes, distributed data-parallel over 8 NeuronCores.

Math: exp(-dist2) = exp(2*S + bias) where
  S[n,m] = cross[n,m] - p_sq[m]/2  (cross via bf16 hi/lo split GEMMs)
  bias[n] = -x_sq[n]               (f32, per-partition bias of the ACT exp)

bf16 split: x = xh + xl, p = ph + pl (each bf16). cross ~= xh@ph + xh@pl
+ xl@ph (xl@pl ~ 2^-18, dropped). Host ships xhl=[xh|xl] as one [N,128]
bf16 tensor; ONE PE transpose per tile gives T=[xh_t; xl_t] stacked.
  mm1: [xh_t; 1; 1].T @ [ph; -psq_h/2; -psq_l/2]   (K=66)
  mm2: [xh_t; xl_t].T @ [pl; ph]                   (K=128, accumulates)
Then ACT exp(2*psum - x_sq) -> SBUF -> DMA out. All engines well under the
~400 GB/s-per-core DMA roofline (~38 MB traffic -> ~105 us/core).
"""

import numpy as np

# Problem constants (hardcoded per harness contract; kernel.py is self-contained)
N = 131072
D = 64
M = 512
GAMMA = 1.0
NCORES = 8
NSHARD = N // NCORES  # 16384
P = 128
LHS_SLOTS = 4  # manual rotation slots for A (ones rows initialized once)

_cache = {}


def _build_bass(nshard=NSHARD):
    import concourse.mybir as mybir
    import concourse.tile as tile
    from concourse import bacc
    from concourse.masks import make_identity

    f32 = mybir.dt.float32
    bf16 = mybir.dt.bfloat16
    nt = nshard // P

    nc = bacc.Bacc(None, target_bir_lowering=False)
    xhl_d = nc.dram_tensor("xhl", [nshard, 2 * D], bf16, kind="ExternalInput")
    nxsq_d = nc.dram_tensor("nxsq", [nshard, 1], f32, kind="ExternalInput")
    rhs1_d = nc.dram_tensor("rhs1", [D + 2, M], bf16, kind="ExternalInput")
    rhs2_d = nc.dram_tensor("rhs2", [2 * D, M], bf16, kind="ExternalInput")
    out_d = nc.dram_tensor("out", [nshard, M], f32, kind="ExternalOutput")

    with tile.TileContext(nc) as tc:
        with (
            tc.tile_pool(name="singles", bufs=1) as singles,
            tc.tile_pool(name="xin", bufs=6) as xin,
            tc.tile_pool(name="tpool", bufs=4) as tpool,
            tc.tile_pool(name="biasp", bufs=6) as biasp,
            tc.tile_pool(name="outp", bufs=4) as outp,
            tc.tile_pool(name="ps_t", bufs=2, space="PSUM") as ps_t,
            tc.tile_pool(name="ps_o", bufs=4, space="PSUM") as ps_o,
        ):
            rhs1_sb = singles.tile([D + 2, M], bf16)
            nc.sync.dma_start(rhs1_sb[:], rhs1_d[:])
            rhs2_sb = singles.tile([2 * D, M], bf16)
            nc.sync.dma_start(rhs2_sb[:], rhs2_d[:])

            ident = singles.tile([P, P], bf16)
            make_identity(nc, ident[:])

            # A slots [66, 128]: rows 0..63 = xh_t (copied per tile), rows
            # 64..65 = ones, initialized once per slot.
            a_slots = []
            for j in range(LHS_SLOTS):
                A_sb = singles.tile([D + 2, P], bf16, name=f"A{j}")
                nc.vector.memset(A_sb[D : D + 2, :], 1.0)
                a_slots.append(A_sb)

            for i in range(nt):
                X_sb = xin.tile([P, 2 * D], bf16, tag="X")
                nc.sync.dma_start(X_sb[:], xhl_d[i * P : (i + 1) * P, :])
                nxsq_sb = biasp.tile([P, 1], f32, tag="nxsq")
                nc.sync.dma_start(nxsq_sb[:], nxsq_d[i * P : (i + 1) * P, :])

                # One transpose gives T = [xh_t; xl_t] [128, 128]
                T_ps = ps_t.tile([P, P], bf16, tag="T_ps")
                nc.tensor.transpose(T_ps[:], X_sb[:], ident[:])
                T = tpool.tile([P, P], bf16, tag="T")
                nc.vector.tensor_copy(T[:], T_ps[:])

                A = a_slots[i % LHS_SLOTS]
                nc.vector.tensor_copy(A[0:D, :], T[0:D, :])

                psum = ps_o.tile([P, M], f32, tag="psum")
                nc.tensor.matmul(psum[:], A[:], rhs1_sb[:], start=True, stop=False)
                nc.tensor.matmul(psum[:], T[:], rhs2_sb[:], start=False, stop=True)

                # out = exp(2*S - x_sq)
                o_sb = outp.tile([P, M], f32, tag="o")
                nc.scalar.activation(
                    o_sb[:],
                    psum[:],
                    mybir.ActivationFunctionType.Exp,
                    bias=nxsq_sb[:],
                    scale=2.0,
                )

                nc.sync.dma_start(out_d[i * P : (i + 1) * P, :], o_sb[:])

    nc.finalize()
    return nc


def _get_nc():
    if "nc" not in _cache:
        _cache["nc"] = _build_bass()
    return _cache["nc"]


def _prep_inputs(x, prototypes):
    import ml_dtypes

    bf = ml_dtypes.bfloat16
    x = np.ascontiguousarray(np.asarray(x, dtype=np.float32))
    prototypes = np.ascontiguousarray(np.asarray(prototypes, dtype=np.float32))

    xh = x.astype(bf)
    xl = (x - xh.astype(np.float32)).astype(bf)
    xhl = np.ascontiguousarray(np.concatenate([xh, xl], axis=1))  # [N, 128] bf16

    nxsq = -(x.astype(np.float64) ** 2).sum(axis=1, keepdims=True)
    nxsq = np.ascontiguousarray(nxsq.astype(np.float32))  # [N, 1] f32

    pt = prototypes.T.astype(np.float32)  # [64, 512]
    ph = pt.astype(bf)
    pl = (pt - ph.astype(np.float32)).astype(bf)

    p_sq = (prototypes.astype(np.float64) ** 2).sum(axis=1)  # [512]
    t = (-0.5 * p_sq).astype(np.float32)
    th = t.astype(bf)
    tl = (t - th.astype(np.float32)).astype(bf)

    rhs1 = np.ascontiguousarray(
        np.concatenate([ph, th[None, :], tl[None, :]], axis=0)
    )  # [66, 512] bf16
    rhs2 = np.ascontiguousarray(np.concatenate([pl, ph], axis=0))  # [128, 512]

    in_maps = []
    for s in range(NCORES):
        sl = slice(s * NSHARD, (s + 1) * NSHARD)
        in_maps.append(
            {
                "xhl": np.ascontiguousarray(xhl[sl]),
                "nxsq": np.ascontiguousarray(nxsq[sl]),
                "rhs1": rhs1,
                "rhs2": rhs2,
            }
        )
    return in_maps


def _run(inputs, trace=False):
    from concourse.bass_utils import run_bass_kernel_spmd

    in_maps = _prep_inputs(inputs["x"], inputs["prototypes"])
    nc = _get_nc()
    res = run_bass_kernel_spmd(
        nc, in_maps, core_ids=list(range(NCORES)), trace=trace
    )
    out = np.concatenate([r["out"] for r in res.results], axis=0)
    return out, res


def kernel(**inputs) -> np.ndarray:
    out, _ = _run(inputs, trace=False)
    return out


# revision 9
# speedup vs baseline: 1.8201x; 1.8201x over previous
"""RBF kernel layer (retrieval_knn): out = exp(-||x - p||^2) for x [131072, 64]
against 512 prototypes, distributed data-parallel over 8 NeuronCores.

Math: exp(-dist2) = exp(2*S + bias) where
  S[n,m] = cross[n,m] - p_sq[m]/2  (cross via bf16 hi/lo split GEMMs)
  bias[n] = -x_sq[n]               (f32, per-partition bias of the ACT exp)

bf16 split: x = xh + xl, p = ph + pl (each bf16). cross ~= xh@ph + xh@pl
+ xl@ph (xl@pl ~ 2^-18, dropped). Host ships xhl=[xh|xl] packed
column-blocked ([128, nt*128] bf16) so one DMA loads XCHUNK tiles; ONE PE
transpose per tile gives T=[xh_t; xl_t] stacked.
  mm1: [xh_t; 1; 1].T @ [ph; -psq_h/2; -psq_l/2]   (K=66, start)
  mm2: [xh_t; xl_t].T @ [pl; ph]                   (K=128, accumulate)
Then ACT exp(2*psum - x_sq) -> SBUF; output DMAs batched OCHUNK tiles.
DMA instruction count is minimized because each HWDGE dma_start costs the
Sync engine ~600 ns of descriptor generation.
"""

import numpy as np

# Problem constants (hardcoded per harness contract; kernel.py is self-contained)
N = 131072
D = 64
M = 512
GAMMA = 1.0
NCORES = 8
NSHARD = N // NCORES  # 16384
P = 128
LHS_SLOTS = 4  # manual rotation slots for A (ones rows initialized once)
XCHUNK = 4  # x tiles per input DMA
OCHUNK = 2  # output tiles per output DMA

_cache = {}


def _build_bass(nshard=NSHARD):
    import concourse.mybir as mybir
    import concourse.tile as tile
    from concourse import bacc
    from concourse.masks import make_identity

    f32 = mybir.dt.float32
    bf16 = mybir.dt.bfloat16
    nt = nshard // P
    assert nt % XCHUNK == 0 and nt % OCHUNK == 0

    nc = bacc.Bacc(None, target_bir_lowering=False)
    # column-blocked: [p, i*128+c] = xhl[i*P + p, c]
    xhl_d = nc.dram_tensor("xhl", [P, nt * 2 * D], bf16, kind="ExternalInput")
    # [p, i] = -x_sq[i*P + p]
    nxsq_d = nc.dram_tensor("nxsq", [P, nt], f32, kind="ExternalInput")
    rhs1_d = nc.dram_tensor("rhs1", [D + 2, M], bf16, kind="ExternalInput")
    rhs2_d = nc.dram_tensor("rhs2", [2 * D, M], bf16, kind="ExternalInput")
    out_d = nc.dram_tensor("out", [nshard, M], f32, kind="ExternalOutput")

    with tile.TileContext(nc) as tc:
        with (
            tc.tile_pool(name="singles", bufs=1) as singles,
            tc.tile_pool(name="xin", bufs=3) as xin,
            tc.tile_pool(name="tpool", bufs=4) as tpool,
            tc.tile_pool(name="outp", bufs=3) as outp,
            tc.tile_pool(name="ps_t", bufs=2, space="PSUM") as ps_t,
            tc.tile_pool(name="ps_o", bufs=4, space="PSUM") as ps_o,
        ):
            rhs1_sb = singles.tile([D + 2, M], bf16)
            nc.sync.dma_start(rhs1_sb[:], rhs1_d[:])
            rhs2_sb = singles.tile([2 * D, M], bf16)
            nc.sync.dma_start(rhs2_sb[:], rhs2_d[:])
            nxsq_sb = singles.tile([P, nt], f32)
            nc.sync.dma_start(nxsq_sb[:], nxsq_d[:])

            ident = singles.tile([P, P], bf16)
            make_identity(nc, ident[:])

            # A slots [66, 128]: rows 0..63 = xh_t (copied per tile), rows
            # 64..65 = ones, initialized once per slot.
            a_slots = []
            for j in range(LHS_SLOTS):
                A_sb = singles.tile([D + 2, P], bf16, name=f"A{j}")
                nc.vector.memset(A_sb[D : D + 2, :], 1.0)
                a_slots.append(A_sb)

            for c in range(nt // XCHUNK):
                X_sb = xin.tile([P, XCHUNK * P], bf16, tag="X")
                nc.sync.dma_start(
                    X_sb[:], xhl_d[:, c * XCHUNK * P : (c + 1) * XCHUNK * P]
                )
                for j in range(XCHUNK):
                    i = c * XCHUNK + j
                    k = i % OCHUNK
                    if k == 0:
                        o_sb = outp.tile([P, OCHUNK, M], f32, tag="o")

                    # One transpose gives T = [xh_t; xl_t] [128, 128]
                    T_ps = ps_t.tile([P, P], bf16, tag="T_ps")
                    nc.tensor.transpose(
                        T_ps[:], X_sb[:, j * P : (j + 1) * P], ident[:]
                    )
                    T = tpool.tile([P, P], bf16, tag="T")
                    nc.vector.tensor_copy(T[:], T_ps[:])

                    A = a_slots[i % LHS_SLOTS]
                    nc.vector.tensor_copy(A[0:D, :], T[0:D, :])

                    psum = ps_o.tile([P, M], f32, tag="psum")
                    nc.tensor.matmul(
                        psum[:], A[:], rhs1_sb[:], start=True, stop=False
                    )
                    nc.tensor.matmul(
                        psum[:], T[:], rhs2_sb[:], start=False, stop=True
                    )

                    # out = exp(2*S - x_sq)
                    nc.scalar.activation(
                        o_sb[:, k, :],
                        psum[:],
                        mybir.ActivationFunctionType.Exp,
                        bias=nxsq_sb[:, i : i + 1],
                        scale=2.0,
                    )

                    if k == OCHUNK - 1:
                        i0 = i - (OCHUNK - 1)
                        dest = out_d[i0 * P : (i0 + OCHUNK) * P, :].rearrange(
                            "(t p) m -> p t m", t=OCHUNK
                        )
                        nc.sync.dma_start(dest, o_sb[:])

    nc.finalize()
    return nc


def _get_nc():
    if "nc" not in _cache:
        _cache["nc"] = _build_bass()
    return _cache["nc"]


def _prep_core_arrays(x, prototypes, nshard):
    """Build full-problem host arrays (xhl_r, nxsq_r per shard; rhs1/rhs2)."""
    import ml_dtypes

    bf = ml_dtypes.bfloat16
    x = np.ascontiguousarray(np.asarray(x, dtype=np.float32))
    prototypes = np.ascontiguousarray(np.asarray(prototypes, dtype=np.float32))

    xh = x.astype(bf)
    xl = (x - xh.astype(np.float32)).astype(bf)
    xhl = np.concatenate([xh, xl], axis=1)  # [N, 128] bf16

    nxsq = -(x.astype(np.float64) ** 2).sum(axis=1)
    nxsq = nxsq.astype(np.float32)  # [N]

    pt = prototypes.T.astype(np.float32)  # [64, 512]
    ph = pt.astype(bf)
    pl = (pt - ph.astype(np.float32)).astype(bf)

    p_sq = (prototypes.astype(np.float64) ** 2).sum(axis=1)  # [512]
    t = (-0.5 * p_sq).astype(np.float32)
    th = t.astype(bf)
    tl = (t - th.astype(np.float32)).astype(bf)

    rhs1 = np.ascontiguousarray(
        np.concatenate([ph, th[None, :], tl[None, :]], axis=0)
    )  # [66, 512] bf16
    rhs2 = np.ascontiguousarray(np.concatenate([pl, ph], axis=0))  # [128, 512]

    nt = nshard // P
    ncores = x.shape[0] // nshard
    in_maps = []
    for s in range(ncores):
        sl = slice(s * nshard, (s + 1) * nshard)
        # column-blocked layouts
        xhl_r = np.ascontiguousarray(
            xhl[sl].reshape(nt, P, 2 * D).transpose(1, 0, 2).reshape(P, nt * 2 * D)
        )
        nxsq_r = np.ascontiguousarray(nxsq[sl].reshape(nt, P).T)
        in_maps.append(
            {"xhl": xhl_r, "nxsq": nxsq_r, "rhs1": rhs1, "rhs2": rhs2}
        )
    return in_maps


def _prep_inputs(x, prototypes):
    return _prep_core_arrays(x, prototypes, NSHARD)


def _run(inputs, trace=False):
    from concourse.bass_utils import run_bass_kernel_spmd

    in_maps = _prep_inputs(inputs["x"], inputs["prototypes"])
    nc = _get_nc()
    res = run_bass_kernel_spmd(
        nc, in_maps, core_ids=list(range(NCORES)), trace=trace
    )
    out = np.concatenate([r["out"] for r in res.results], axis=0)
    return out, res


def kernel(**inputs) -> np.ndarray:
    out, _ = _run(inputs, trace=False)
    return out


# revision 13
# speedup vs baseline: 2.0574x; 1.1304x over previous
"""RBF kernel layer (retrieval_knn): out = exp(-||x - p||^2) for x [131072, 64]
against 512 prototypes, distributed data-parallel over 8 NeuronCores.

Math: exp(-dist2) = exp(2*S) where S[n,m] = cross[n,m] - p_sq[m]/2 - x_sq[n]/2,
computed entirely in two bf16 hi/lo-split GEMMs accumulating in fp32 PSUM:
  mm1: [xh_t; 1; 1; nxsq_h; nxsq_l].T @ [ph; npsq_h; npsq_l; 1; 1]  (K=68)
  mm2: [xh_t; xl_t].T @ [pl; ph]                                    (K=128)
where x = xh + xl, p = ph + pl (bf16 splits; the dropped xl@pl term is
~2^-18), npsq* = bf16 split of -p_sq/2, nxsq* = bf16 split of -x_sq/2.

Host ships xhl=[xh|xl] column-blocked ([128, nt*128] bf16) so one DMA loads
XCHUNK tiles; ONE PE transpose per tile gives T=[xh_t; xl_t]. The exp has no
per-tile bias, so one ACTIVATE covers OCHUNK tiles' PSUM banks and one DMA
stores OCHUNK tiles. DMA instruction count is minimized because each HWDGE
dma_start costs the issuing engine ~600 ns of descriptor generation.
"""

import numpy as np

# Problem constants (hardcoded per harness contract; kernel.py is self-contained)
N = 131072
D = 64
M = 512
GAMMA = 1.0
NCORES = 8
NSHARD = N // NCORES  # 16384
P = 128
K1 = D + 4  # mm1 contraction: 64 x rows + 2 psq rows + 2 xsq rows
LHS_SLOTS = 4  # manual rotation slots for A (ones rows initialized once)
XCHUNK = 8  # x tiles per input DMA
OCHUNK = 2  # output tiles per ACTIVATE + output DMA (PSUM pair)

_cache = {}


def _build_bass(nshard=NSHARD):
    import concourse.mybir as mybir
    import concourse.tile as tile
    from concourse import bacc
    from concourse.masks import make_identity

    f32 = mybir.dt.float32
    bf16 = mybir.dt.bfloat16
    nt = nshard // P
    assert nt % XCHUNK == 0 and XCHUNK % OCHUNK == 0

    nc = bacc.Bacc(None, target_bir_lowering=False)
    # column-blocked: [p, i*128+c] = xhl[i*P + p, c]
    xhl_d = nc.dram_tensor("xhl", [P, nt * 2 * D], bf16, kind="ExternalInput")
    # rows (h, l) of -x_sq/2 in bf16, column-blocked like xhl
    nxsq_d = nc.dram_tensor("nxsq", [2, nt * P], bf16, kind="ExternalInput")
    rhs1_d = nc.dram_tensor("rhs1", [K1, M], bf16, kind="ExternalInput")
    rhs2_d = nc.dram_tensor("rhs2", [2 * D, M], bf16, kind="ExternalInput")
    out_d = nc.dram_tensor("out", [nshard, M], f32, kind="ExternalOutput")

    with tile.TileContext(nc) as tc:
        with (
            tc.tile_pool(name="singles", bufs=1) as singles,
            tc.tile_pool(name="xin", bufs=3) as xin,
            tc.tile_pool(name="tpool", bufs=4) as tpool,
            tc.tile_pool(name="outp", bufs=3) as outp,
            tc.tile_pool(name="ps_t", bufs=2, space="PSUM") as ps_t,
            tc.tile_pool(name="ps_o", bufs=3, space="PSUM") as ps_o,
        ):
            rhs1_sb = singles.tile([K1, M], bf16)
            nc.sync.dma_start(rhs1_sb[:], rhs1_d[:])
            rhs2_sb = singles.tile([2 * D, M], bf16)
            nc.sync.dma_start(rhs2_sb[:], rhs2_d[:])
            nxsq_sb = singles.tile([2, nt * P], bf16)
            nc.sync.dma_start(nxsq_sb[:], nxsq_d[:])

            ident = singles.tile([P, P], bf16)
            make_identity(nc, ident[:])

            # A slots [68, 128]: rows 0..63 = xh_t, 64..65 = -x_sq/2 hi/lo
            # (copied per tile; start partition 64 is AP-legal), 66..67 =
            # ones (static; initialized by the one-time memset of 64..68).
            a_slots = []
            for j in range(LHS_SLOTS):
                A_sb = singles.tile([K1, P], bf16, name=f"A{j}")
                nc.vector.memset(A_sb[D:K1, :], 1.0)
                a_slots.append(A_sb)

            for c in range(nt // XCHUNK):
                X_sb = xin.tile([P, XCHUNK * P], bf16, tag="X")
                nc.sync.dma_start(
                    X_sb[:], xhl_d[:, c * XCHUNK * P : (c + 1) * XCHUNK * P]
                )
                for j in range(XCHUNK):
                    i = c * XCHUNK + j
                    k = i % OCHUNK
                    if k == 0:
                        o_sb = outp.tile([P, OCHUNK, M], f32, tag="o")
                        psum = ps_o.tile([P, OCHUNK, M], f32, tag="psum")

                    # One transpose gives T = [xh_t; xl_t] [128, 128]
                    T_ps = ps_t.tile([P, P], bf16, tag="T_ps")
                    nc.tensor.transpose(
                        T_ps[:], X_sb[:, j * P : (j + 1) * P], ident[:]
                    )
                    T = tpool.tile([P, P], bf16, tag="T")
                    nc.vector.tensor_copy(T[:], T_ps[:])

                    A = a_slots[i % LHS_SLOTS]
                    nc.vector.tensor_copy(A[0:D, :], T[0:D, :])
                    nc.vector.tensor_copy(
                        A[D : D + 2, :], nxsq_sb[:, i * P : (i + 1) * P]
                    )

                    nc.tensor.matmul(
                        psum[:, k, :], A[:], rhs1_sb[:], start=True, stop=False
                    )
                    nc.tensor.matmul(
                        psum[:, k, :], T[:], rhs2_sb[:], start=False, stop=True
                    )

                    if k == OCHUNK - 1:
                        # out = exp(2*S) over both PSUM banks at once
                        nc.scalar.activation(
                            o_sb[:],
                            psum[:],
                            mybir.ActivationFunctionType.Exp,
                            bias=0.0,
                            scale=2.0,
                        )
                        i0 = i - (OCHUNK - 1)
                        dest = out_d[i0 * P : (i0 + OCHUNK) * P, :].rearrange(
                            "(t p) m -> p t m", t=OCHUNK
                        )
                        nc.sync.dma_start(dest, o_sb[:])

    nc.finalize()
    return nc


def _get_nc():
    if "nc" not in _cache:
        _cache["nc"] = _build_bass()
    return _cache["nc"]


def _prep_core_arrays(x, prototypes, nshard):
    """Build per-core host arrays (xhl, nxsq column-blocked; rhs1/rhs2)."""
    import ml_dtypes

    bf = ml_dtypes.bfloat16
    x = np.ascontiguousarray(np.asarray(x, dtype=np.float32))
    prototypes = np.ascontiguousarray(np.asarray(prototypes, dtype=np.float32))

    xh = x.astype(bf)
    xl = (x - xh.astype(np.float32)).astype(bf)
    xhl = np.concatenate([xh, xl], axis=1)  # [N, 128] bf16

    nxsq = (-0.5 * (x.astype(np.float64) ** 2).sum(axis=1)).astype(np.float32)
    nxh = nxsq.astype(bf)
    nxl = (nxsq - nxh.astype(np.float32)).astype(bf)

    pt = prototypes.T.astype(np.float32)  # [64, 512]
    ph = pt.astype(bf)
    pl = (pt - ph.astype(np.float32)).astype(bf)

    p_sq = (prototypes.astype(np.float64) ** 2).sum(axis=1)  # [512]
    t = (-0.5 * p_sq).astype(np.float32)
    th = t.astype(bf)
    tl = (t - th.astype(np.float32)).astype(bf)

    ones = np.ones((1, M), dtype=bf)
    # row order matches A: [xh_t rows; nxsq h/l rows; ones rows]
    rhs1 = np.ascontiguousarray(
        np.concatenate([ph, ones, ones, th[None, :], tl[None, :]], axis=0)
    )  # [68, 512] bf16
    rhs2 = np.ascontiguousarray(np.concatenate([pl, ph], axis=0))  # [128, 512]

    nt = nshard // P
    ncores = x.shape[0] // nshard
    in_maps = []
    for s in range(ncores):
        sl = slice(s * nshard, (s + 1) * nshard)
        xhl_r = np.ascontiguousarray(
            xhl[sl].reshape(nt, P, 2 * D).transpose(1, 0, 2).reshape(P, nt * 2 * D)
        )
        nxsq_r = np.ascontiguousarray(
            np.stack([nxh[sl], nxl[sl]], axis=0)
        )  # [2, nshard] already tile-column-blocked ([2, i*P+p])
        in_maps.append(
            {"xhl": xhl_r, "nxsq": nxsq_r, "rhs1": rhs1, "rhs2": rhs2}
        )
    return in_maps


def _prep_inputs(x, prototypes):
    return _prep_core_arrays(x, prototypes, NSHARD)


def _run(inputs, trace=False):
    from concourse.bass_utils import run_bass_kernel_spmd

    in_maps = _prep_inputs(inputs["x"], inputs["prototypes"])
    nc = _get_nc()
    res = run_bass_kernel_spmd(
        nc, in_maps, core_ids=list(range(NCORES)), trace=trace
    )
    out = np.concatenate([r["out"] for r in res.results], axis=0)
    return out, res


def kernel(**inputs) -> np.ndarray:
    out, _ = _run(inputs, trace=False)
    return out
